# revision 74
# baseline (speedup 1.0000x reference)
"""CTGRU forward kernel for one TRN2 chip (8 NeuronCores, data-parallel).

v4 layout strategy (per core, batch shard BC=512):
  - Transposed gate matmuls: feature j on partitions, batch b on free.
  - Plane truncation MKR/MKS=3 (of M=8): softmax logits -(z-c_m)^2 with
    c_m = m*0.5*ln10 make high planes negligible; numpy-sim validated as
    numerically identical to MK=4 (rel err 1.24e-2 either way).
  - fp8 (e4m3) DoubleRow matmuls for the two big gates (weights x16,
    host-quantized; x8/h8 moving operands). The q gate stays FULL bf16:
    any fp8 on the q path (x, ctx, or Wq) blows rel err to ~4e-2 because
    q writes the state directly, while the softmax gates are noise
    tolerant.  NB: fp8 DR costs the same ~379ns/inst as bf16 on this HW;
    its only win is halved instruction count.
  - One-op Gaussian: ACT Derivative_Erf(x) = 2/sqrt(pi)*exp(-x^2); the
    constant cancels in softmax ratios.
  - Reciprocals on ACT (raw InstActivation Reciprocal, ~1.2e-5 max err),
    merged into single 4-plane calls per gate: the Tile scheduler
    interleaves ready ACTs so fewer calls = fewer 1283ns table reloads
    (11 -> 5 per step). DVE InstReciprocal is 12x slower than a tensor
    op; AluOp divide + custom DVE ops do not compile on this toolchain.
  - u1 = q - h_hat on DVE (pool Q7 launches stretch concurrent DVE ops
    up to 2.6x via SBUF contention; keep GpSimd idle).
  - SORDER=all: s-gate matmuls for all 4 u-blocks run before the q gate,
    covering the er->den->recip->ctx critical chain; QSPLIT: q's x-part
    k-tiles for all 4 u-blocks run before any ctx-part k-tile.
  - t=0 specialization: h=0 so the r gate is skipped entirely (ctx=0),
    s/q matmuls contract x k-tiles only, state update is hh = s*q*D.
  - State h_hat: [128, NG, MKS, BC] bf16; gate DVE work done pair-wide
    over [128, MK*BC] slices with stride-0 middle-dim broadcast APs.
  - All weights resident in SBUF; x8/xbf double-buffered DMA per step.
Best measured: 795907ns (vs 2723402 session baseline), rel err 1.235e-2.
"""

import os
import sys

import numpy as np
import ml_dtypes

for _p in ("/root/.axon_site/_ro/trn_rl_repo", "/opt/trn_rl_repo"):
    if os.path.isdir(_p) and _p not in sys.path:
        sys.path.append(_p)

import concourse.bass as bass
import concourse.tile as tile
from concourse import mybir
from concourse.bass import AP
from concourse.bass_utils import run_bass_kernel_spmd
from concourse.masks import make_identity

BF16 = mybir.dt.bfloat16
F32 = mybir.dt.float32
E4 = mybir.dt.float8e4
NPBF16 = ml_dtypes.bfloat16
NPE4 = ml_dtypes.float8_e4m3
AF = mybir.ActivationFunctionType
PM = mybir.MatmulPerfMode

B, T, F, U, M = 4096, 16, 512, 512, 8
OUT = 3
NCORES = 8
BC = B // NCORES          # batch per core
NG = U // 128             # u-blocks (4)
NKT = (F + U) // 128      # k-tiles of fused input (8)
NXT = F // 128            # x k-tiles (4)
MKR = int(os.environ.get("K_MKR", "3"))  # r-gate planes kept
MKS = int(os.environ.get("K_MKS", "3"))  # s-gate planes kept
NJR = NG * MKR
NJS = NG * MKS
DELTA_T = 0.04
WSC = 16.0                # big-gate weight scale before e4m3 quantization
QSC = 256.0               # q-gate weight scale before e4m3 quantization

QMODE = os.environ.get("K_QMODE", "bf16")      # "bf16" | "mixed" | "fp8"
QFP8 = QMODE == "fp8"
RECIP = os.environ.get("K_RECIP", "act")       # "act" | "div" | "fast" | "dve"
RDT_IS_F32 = RECIP == "fast"
SUB_MODE = os.environ.get("K_SUB_MODE", "dve_bc")  # "dve_bc" | "pool_bc"
SORDER = os.environ.get("K_SORDER", "all")     # "split" | "all"
QSPLIT = os.environ.get("K_QSPLIT", "1") == "1"  # q: all x-parts before ctx-parts
H8ENG = os.environ.get("K_H8", "act")          # "act" | "pool" | "dve"
H8M = os.environ.get("K_H8M", "0") == "1"      # single merged h8 copy
OCOPY = os.environ.get("K_OCOPY", "act")       # "act" | "dve"
RECIP1 = os.environ.get("K_RECIP1", "1") == "1"  # single 4-plane recip calls
RRECIP = os.environ.get("K_RRECIP", "merged")  # "merged" | "split" (r gate only)
OUTDEFER = os.environ.get("K_OUTDEFER", "0") == "1"  # out-gate lags one step
WIDE = os.environ.get("K_WIDE", "0") == "1"    # all-g fused consume chain
# Paired ACTs: one activation per (g-pair, m) over a 2-bank PSUM tile.
# Valid only when br/bs/bq are all zero (bias is then plane-constant);
# checked at kernel() time. "auto" enables iff biases are zero.
PAIRQ = os.environ.get("K_PAIR", "0")          # "auto" | "1" | "0"
# Emit r/s recips after ALL es ACTs: keeps the D_ERF stream unsplit and
# makes the two RECIP activations adjacent (single table load).
LATE_RECIP = os.environ.get("K_LATE_RECIP", "1") == "1"
# Hoist w = es*rs ahead of the q gate (possible since LATE_RECIP makes
# r_s available pre-tanh): shortens the post-tanh consume chain.
WHOIST = os.environ.get("K_WHOIST", "0") == "1"
# Dummy D_ERF right after the tanhs: preloads the erf table during the
# post-tanh scalar idle window so the next step's er stream skips its
# 1.28us table load (only table-neutral COPYs run in between).
DPRE = os.environ.get("K_DPRE", "0") == "1"
# (measured: pair=830961ns vs split ACTs 779364ns — coarser matmul->ACT
#  granularity and 2-bank PSUM tiles cost more than the ACT savings)
T0_SPECIAL = os.environ.get("K_T0", "1") == "1"

_LN_TAU = (np.arange(M) * (0.5 * np.log(10.0))).astype(np.float64)
DECAY = np.exp(-DELTA_T / (np.exp(_LN_TAU) + 1e-7)).astype(np.float32)
LN_TAU = _LN_TAU.astype(np.float32)


def _split_sync_waits(nc, max_waits=1):
    """walrus (CoreV3) accepts at most one sync-wait command per
    instruction; hoist extras onto NoOps placed just before."""
    n = 0
    for fn in nc.m.functions:
        for bb in fn.blocks:
            new_list = []
            for inst in bb.instructions:
                si = inst.sync_info
                if si is not None and si.on_wait and len(si.on_wait) > max_waits:
                    waits = list(si.on_wait)
                    extra, keep = waits[:-max_waits], waits[-max_waits:]
                    for i in range(0, len(extra), max_waits):
                        nop = mybir.InstNoOp(name=f"{inst.name}-wsplit{n}")
                        nop.engine = inst.engine
                        nop.sync_info = mybir.SyncInfo(
                            on_wait=extra[i : i + max_waits], on_update=[]
                        )
                        new_list.append(nop)
                        n += 1
                    si.on_wait = keep
                new_list.append(inst)
            bb.instructions[:] = new_list
    return n


def _act_reciprocal(nc, out, in_):
    """InstActivation(Reciprocal) emitted directly; measured max rel err on
    this toolchain is 1.2e-5 — far below this kernel's bf16 noise floor."""
    eng = nc.scalar
    ins = [eng.lower_ap(in_)]
    for arg in (0.0, 1.0, 0.0):  # bias, scale, alpha
        ins.append(mybir.ImmediateValue(dtype=mybir.dt.float32, value=arg))
    return eng.add_instruction(
        mybir.InstActivation(
            name=nc.get_next_instruction_name(),
            func=mybir.ActivationFunctionType.Reciprocal,
            ins=ins,
            outs=[eng.lower_ap(out)],
        )
    )


def _recip(nc, out, in_):
    if RECIP == "fast":
        # in-place 1/x on the f32 den tile; ~51 ULP, single DVE op.
        # den = sum of exp(-(z-c)^2) terms is always normal-range positive.
        nc.vector.reciprocal_approx_fast(out=out, in_=in_)
    elif RECIP == "dve":
        with nc.allow_low_precision("bf16 softmax denominators"):
            nc.vector.reciprocal(out, in_)
    else:
        _act_reciprocal(nc, out, in_)


def _bcast_mid(ap2d, n):
    """[128, BC] AP -> [128, n, BC] with stride-0 middle dim (read b'cast)."""
    return AP(ap2d.tensor, ap2d.offset, [ap2d.ap[0], [0, n], ap2d.ap[1]])


def _tree_sum(nc, dst, planes, mk, tmp_pool, tag):
    """dst[128, BC] = sum of planes[:, 0:mk, :] via pairwise DVE adds."""
    if mk == 2:
        nc.vector.tensor_add(dst, planes[:, 0, :], planes[:, 1, :])
    elif mk == 3:
        t1 = tmp_pool.tile([128, BC], BF16, tag=tag)
        nc.vector.tensor_add(t1, planes[:, 0, :], planes[:, 1, :])
        nc.vector.tensor_add(dst, t1, planes[:, 2, :])
    elif mk == 4:
        t1 = tmp_pool.tile([128, 2, BC], BF16, tag=tag)
        nc.vector.tensor_add(t1, planes[:, 0:2, :], planes[:, 2:4, :])
        nc.vector.tensor_add(dst, t1[:, 0, :], t1[:, 1, :])
    else:
        raise ValueError(mk)


def build_program(t_steps=T, pair=False):
    nc = bass.Bass()
    xT8_d = nc.declare_dram_parameter("xT8", [t_steps, F, BC], E4, isOutput=False)
    if not QFP8:
        xT_d = nc.declare_dram_parameter("xT", [t_steps, F, BC], BF16, isOutput=False)
    wr_d = nc.declare_dram_parameter("wr", [F + U, NJR * 128], E4, isOutput=False)
    ws_d = nc.declare_dram_parameter("ws", [F + U, NJS * 128], E4, isOutput=False)
    wq_d = nc.declare_dram_parameter("wq", [F + U, U], E4 if QFP8 else BF16,
                                     isOutput=False)
    wo_d = nc.declare_dram_parameter("wo", [U, OUT], BF16, isOutput=False)
    rb_d = nc.declare_dram_parameter("rbias", [128, NJR], F32, isOutput=False)
    sb_d = nc.declare_dram_parameter("sbias", [128, NJS], F32, isOutput=False)
    qb_d = nc.declare_dram_parameter("qbias", [128, NG], F32, isOutput=False)
    out_d = nc.declare_dram_parameter("out", [BC, t_steps, OUT], F32, isOutput=True)

    with tile.TileContext(nc) as tc:
        from contextlib import ExitStack

        with ExitStack() as ctx:
            const = ctx.enter_context(tc.tile_pool(name="const", bufs=1))
            p_x = ctx.enter_context(tc.tile_pool(name="xload", bufs=2))
            p_e = ctx.enter_context(
                tc.tile_pool(name="ering",
                             bufs=int(os.environ.get("K_EBUF", "4"))))
            p_es = ctx.enter_context(tc.tile_pool(name="esring", bufs=4))
            p_t = ctx.enter_context(
                tc.tile_pool(name="tmpring",
                             bufs=int(os.environ.get("K_TBUF", "4"))))
            p_n = ctx.enter_context(tc.tile_pool(name="numring", bufs=4))
            p_h = ctx.enter_context(tc.tile_pool(name="hbuf", bufs=2))
            p_cq = ctx.enter_context(tc.tile_pool(name="cq", bufs=2))
            p_d = ctx.enter_context(tc.tile_pool(name="dens", bufs=2))
            p_u = ctx.enter_context(
                tc.tile_pool(name="uring",
                             bufs=int(os.environ.get("K_UVBUF", "4"))))
            p_v = ctx.enter_context(
                tc.tile_pool(name="vring",
                             bufs=int(os.environ.get("K_UVBUF", "4"))))
            p_f = ctx.enter_context(tc.tile_pool(name="f32s", bufs=2))
            p_ps = ctx.enter_context(
                tc.tile_pool(name="ps", space="PSUM",
                             bufs=int(os.environ.get("K_PSB", "5"))))
            if pair:
                p_ps2 = ctx.enter_context(
                    tc.tile_pool(name="ps2", bufs=3, space="PSUM"))
            p_pso = ctx.enter_context(tc.tile_pool(name="pso", bufs=1, space="PSUM"))
            p_pst = ctx.enter_context(tc.tile_pool(name="pst", bufs=1, space="PSUM"))

            # ---- constants / weights ----------------------------------
            wr_sb = const.tile([128, NKT, NJR * 128], E4)
            ws_sb = const.tile([128, NKT, NJS * 128], E4)
            nc.sync.dma_start(
                out=wr_sb, in_=wr_d.rearrange("(kt p) j -> p kt j", p=128)
            )
            nc.sync.dma_start(
                out=ws_sb, in_=ws_d.rearrange("(kt p) j -> p kt j", p=128)
            )
            wq_sb = const.tile([128, NKT, U], E4 if QFP8 else BF16)
            wo_sb = const.tile([128, NG, OUT], BF16)
            rb_sb = const.tile([128, NJR], F32)
            sb_sb = const.tile([128, NJS], F32)
            qb_sb = const.tile([128, NG], F32)
            nc.sync.dma_start(out=wq_sb, in_=wq_d.rearrange("(kt p) j -> p kt j", p=128))
            nc.sync.dma_start(out=wo_sb, in_=wo_d.rearrange("(g p) c -> p g c", p=128))
            nc.sync.dma_start(out=rb_sb, in_=rb_d[:, :])
            nc.sync.dma_start(out=sb_sb, in_=sb_d[:, :])
            nc.sync.dma_start(out=qb_sb, in_=qb_d[:, :])

            hh = const.tile([128, NG, MKS, BC], BF16)    # state
            d4 = const.tile([128, MKS, BC], BF16)        # decay bcast tensor
            o_acc = const.tile([128, NG, t_steps, OUT], F32)
            ident = const.tile([OUT, OUT], F32)
            make_identity(nc, ident)
            nc.vector.memset(hh, 0.0)
            for m in range(MKS):
                nc.vector.memset(d4[:, m, :], float(DECAY[m]))

            h8 = p_h.tile([128, NG, BC], E4, tag="h8")
            nc.vector.memset(h8, 0.0)
            h_cur = None

            def emit_out_gate(t_idx, h_tile):
                pso = p_pso.tile([OUT, BC], F32, tag="pso")
                for g in range(NG):
                    nc.tensor.matmul(
                        pso, wo_sb[:, g, :], h_tile[:, g, :],
                        start=(g == 0), stop=(g == NG - 1))
                oT_t = p_f.tile([OUT, BC], F32, tag="ot")
                nc.scalar.copy(oT_t, pso)
                for bs in range(NG):
                    pst = p_pst.tile([128, OUT], F32, tag="pst")
                    nc.tensor.transpose(
                        pst, oT_t[:, bs * 128 : (bs + 1) * 128], ident
                    )
                    nc.scalar.copy(o_acc[:, bs, t_idx, :], pst)

            def mm_gate(ps, w_sb, jt, xmv, hmv, x_only=False):
                """Accumulate one big-gate plane into psum (fp8 DoubleRow)."""
                np_ = 2 if x_only else NKT // 2
                for p in range(np_):
                    lhs = w_sb[:, 2 * p : 2 * p + 2, jt * 128 : (jt + 1) * 128]
                    rhs = (xmv[:, 2 * p : 2 * p + 2, :] if p < 2
                           else hmv[:, 2 * (p - 2) : 2 * (p - 2) + 2, :])
                    nc.tensor.matmul(ps, lhs, rhs, start=(p == 0),
                                     stop=(p == np_ - 1),
                                     perf_mode=PM.DoubleRow)

            for t in range(t_steps):
                t0 = T0_SPECIAL and t == 0
                # ---- x^T for this step --------------------------------
                x8 = p_x.tile([128, NXT, BC], E4, tag="xt8")
                nc.sync.dma_start(
                    out=x8, in_=xT8_d[t].rearrange("(kt p) b -> p kt b", p=128)
                )
                if not QFP8:
                    xbf = p_x.tile([128, NXT, BC], BF16, tag="xt")
                    nc.sync.dma_start(
                        out=xbf, in_=xT_d[t].rearrange("(kt p) b -> p kt b", p=128)
                    )

                # ---- r gate (skipped at t=0: ctx = 0) -----------------
                if not t0:
                    den_r = p_d.tile([128, NG, BC],
                                     F32 if RDT_IS_F32 else BF16, tag="denr")
                    num_t = []
                    if pair:
                        for gp in range(NG // 2):
                            erp = p_e.tile([128, 2, MKR, BC], BF16, tag="er",
                                           bufs=2)
                            for m in range(MKR):
                                ps2 = p_ps2.tile([128, 2, BC], F32, tag="ps2")
                                mm_gate(ps2[:, 0, :], wr_sb,
                                        (2 * gp) * MKR + m, x8, h8)
                                mm_gate(ps2[:, 1, :], wr_sb,
                                        (2 * gp + 1) * MKR + m, x8, h8)
                                jt0 = (2 * gp) * MKR + m
                                nc.scalar.activation(
                                    erp[:, :, m, :], ps2, AF.Derivative_Erf,
                                    bias=rb_sb[:, jt0 : jt0 + 1],
                                    scale=1.0 / WSC)
                            for i in (0, 1):
                                g = 2 * gp + i
                                er = erp[:, i]
                                _tree_sum(nc, den_r[:, g, :], er, MKR,
                                          p_t, "t1")
                                nc.vector.tensor_mul(er, er, hh[:, g, 0:MKR, :])
                                num = p_n.tile([128, BC], BF16, tag="num")
                                _tree_sum(nc, num, er, MKR, p_t, "t1")
                                num_t.append(num)
                    else:
                      for g in range(NG):
                        er = p_e.tile([128, MKR, BC], BF16, tag="er")
                        for m in range(MKR):
                            jt = g * MKR + m
                            ps = p_ps.tile([128, BC], F32, tag="ps")
                            mm_gate(ps, wr_sb, jt, x8, h8)
                            nc.scalar.activation(
                                er[:, m, :], ps, AF.Derivative_Erf,
                                bias=rb_sb[:, jt : jt + 1], scale=1.0 / WSC)
                        # den tree (keeps er intact), then er <- er*hh
                        _tree_sum(nc, den_r[:, g, :], er, MKR, p_t, "t1")
                        nc.vector.tensor_mul(er, er, hh[:, g, 0:MKR, :])
                        num = p_n.tile([128, BC], BF16, tag="num")
                        _tree_sum(nc, num, er, MKR, p_t, "t1")
                        num_t.append(num)

                    def emit_r_recip():
                        if RDT_IS_F32:
                            r_r = den_r      # reciprocal computed in place
                        else:
                            r_r = p_d.tile([128, NG, BC], BF16, tag="rr")
                        if RECIP1 and RRECIP == "halves":
                            _recip(nc, r_r[:, 0:2, :], den_r[:, 0:2, :])
                            _recip(nc, r_r[:, 2:4, :], den_r[:, 2:4, :])
                        elif RECIP1 and RRECIP == "merged":
                            _recip(nc, r_r, den_r)
                        else:
                            _recip(nc, r_r[:, 0:2, :], den_r[:, 0:2, :])
                            _recip(nc, r_r[:, 2:4, :], den_r[:, 2:4, :])
                        return r_r

                    def emit_ctx(r_r):
                        for g in range(NG):
                            nc.vector.tensor_mul(ctx_t[:, g, :], num_t[g],
                                                 r_r[:, g, :])

                    if QFP8:
                        ctx8 = p_cq.tile([128, NG, BC], E4, tag="ctx8")
                        ctx_t = ctx8
                    else:
                        ctx_t = p_cq.tile([128, NG, BC], BF16, tag="ctx")
                    if RECIP == "div":
                        for g in range(NG):
                            nc.vector.tensor_tensor(
                                ctx_t[:, g, :], num_t[g], den_r[:, g, :],
                                op=mybir.AluOpType.divide)
                    elif not LATE_RECIP:
                        emit_ctx(emit_r_recip())

                # ---- s produce (PE/ACT/DVE-den only) ------------------
                den_s = p_d.tile([128, NG, BC],
                                 F32 if RDT_IS_F32 else BF16, tag="dens")
                es_t = {}
                if WIDE:
                    es_all = p_es.tile([128, NG, MKS, BC], BF16, tag="esw",
                                       bufs=2)

                def s_produce(g):
                    if WIDE:
                        es = es_all[:, g]
                    else:
                        es = p_es.tile([128, MKS, BC], BF16, tag="es")
                    for m in range(MKS):
                        jt = g * MKS + m
                        ps = p_ps.tile([128, BC], F32, tag="ps")
                        mm_gate(ps, ws_sb, jt, x8, h8, x_only=t0)
                        nc.scalar.activation(
                            es[:, m, :], ps, AF.Derivative_Erf,
                            bias=sb_sb[:, jt : jt + 1], scale=1.0 / WSC)
                    _tree_sum(nc, den_s[:, g, :], es, MKS, p_t, "t1s")
                    es_t[g] = es

                def s_produce_pair(gp):
                    esp = p_es.tile([128, 2, MKS, BC], BF16, tag="es", bufs=2)
                    for m in range(MKS):
                        ps2 = p_ps2.tile([128, 2, BC], F32, tag="ps2")
                        mm_gate(ps2[:, 0, :], ws_sb, (2 * gp) * MKS + m,
                                x8, h8, x_only=t0)
                        mm_gate(ps2[:, 1, :], ws_sb, (2 * gp + 1) * MKS + m,
                                x8, h8, x_only=t0)
                        jt0 = (2 * gp) * MKS + m
                        nc.scalar.activation(
                            esp[:, :, m, :], ps2, AF.Derivative_Erf,
                            bias=sb_sb[:, jt0 : jt0 + 1], scale=1.0 / WSC)
                    for i in (0, 1):
                        g = 2 * gp + i
                        _tree_sum(nc, den_s[:, g, :], esp[:, i], MKS,
                                  p_t, "t1s")
                        es_t[g] = esp[:, i]

                if pair:
                    s_produce_pair(0)
                    s_produce_pair(1)
                else:
                    s_produce(0)
                    s_produce(1)
                    if SORDER == "all":
                        s_produce(2)
                        s_produce(3)

                late_r_s = None
                w_pre = None
                if LATE_RECIP and RECIP != "div":
                    # recips emitted after every es ACT: the D_ERF stream
                    # stays unsplit and the two RECIPs share one table load
                    if not t0:
                        rr_late = emit_r_recip()
                    late_r_s = (den_s if RDT_IS_F32 else
                                p_d.tile([128, NG, BC], BF16, tag="rs"))
                    _recip(nc, late_r_s, den_s)
                    if not t0:
                        emit_ctx(rr_late)
                    if WHOIST and not WIDE:
                        # w = es*rs does not depend on q: run it under the
                        # q matmuls instead of inside the consume tail
                        w_pre = []
                        for g in range(NG):
                            v = p_v.tile([128, MKS, BC], BF16, tag="v")
                            nc.vector.tensor_mul(
                                v, es_t[g], _bcast_mid(late_r_s[:, g, :], MKS))
                            w_pre.append(v)

                # ---- q gate -------------------------------------------
                q_t = p_cq.tile([128, NG, BC], BF16, tag="q")
                if pair:
                    # paired tanh over 2-bank PSUM tiles (bq == 0)
                    q_ps = []
                    for gp in range(NG // 2):
                        ps2 = p_ps2.tile([128, 2, BC], F32, tag="ps2")
                        for i in (0, 1):
                            g = 2 * gp + i
                            for kt in range(NXT):
                                nc.tensor.matmul(
                                    ps2[:, i, :],
                                    wq_sb[:, kt, g * 128 : (g + 1) * 128],
                                    xbf[:, kt, :],
                                    start=(kt == 0),
                                    stop=(t0 and kt == NXT - 1))
                        q_ps.append(ps2)
                    for gp in range(NG // 2):
                        ps2 = q_ps[gp]
                        if not t0:
                            for i in (0, 1):
                                g = 2 * gp + i
                                for kt in range(NXT, NKT):
                                    nc.tensor.matmul(
                                        ps2[:, i, :],
                                        wq_sb[:, kt, g * 128 : (g + 1) * 128],
                                        ctx_t[:, kt - 4, :],
                                        start=False, stop=(kt == NKT - 1))
                        nc.scalar.activation(
                            q_t[:, 2 * gp : 2 * gp + 2, :], ps2, AF.Tanh,
                            bias=qb_sb[:, 2 * gp : 2 * gp + 1], scale=1.0)
                elif QSPLIT and not QFP8:
                    # phase 1: x-part k-tiles for all g (no ctx dependency,
                    # keeps PE busy while the den->recip->ctx chain resolves)
                    q_ps = []
                    for g in range(NG):
                        ps = p_ps.tile([128, BC], F32, tag="ps")
                        for kt in range(NXT):
                            nc.tensor.matmul(
                                ps, wq_sb[:, kt, g * 128 : (g + 1) * 128],
                                xbf[:, kt, :],
                                start=(kt == 0), stop=(t0 and kt == NXT - 1))
                        q_ps.append(ps)
                    if OUTDEFER and t > 0:
                        # previous step's out-gate: independent PE work that
                        # fills the stall while the ctx chain resolves
                        emit_out_gate(t - 1, h_prev)
                    # phase 2: ctx-part k-tiles + tanh per g
                    for g in range(NG):
                        ps = q_ps[g]
                        if not t0:
                            for kt in range(NXT, NKT):
                                nc.tensor.matmul(
                                    ps, wq_sb[:, kt, g * 128 : (g + 1) * 128],
                                    ctx_t[:, kt - 4, :],
                                    start=False, stop=(kt == NKT - 1))
                        nc.scalar.activation(
                            q_t[:, g, :], ps, AF.Tanh,
                            bias=qb_sb[:, g : g + 1], scale=1.0)
                else:
                  if OUTDEFER and t > 0:
                      emit_out_gate(t - 1, h_prev)
                  for g in range(NG):
                    ps = p_ps.tile([128, BC], F32, tag="ps")
                    if QFP8:
                        np_ = 2 if t0 else NKT // 2
                        for p in range(np_):
                            lhs = wq_sb[:, 2 * p : 2 * p + 2,
                                        g * 128 : (g + 1) * 128]
                            rhs = (x8[:, 2 * p : 2 * p + 2, :] if p < 2
                                   else ctx8[:, 2 * (p - 2) : 2 * (p - 2) + 2, :])
                            nc.tensor.matmul(ps, lhs, rhs, start=(p == 0),
                                             stop=(p == np_ - 1),
                                             perf_mode=PM.DoubleRow)
                        qscale = 1.0 / QSC
                    else:
                        nkt = NXT if t0 else NKT
                        for kt in range(nkt):
                            rhs = xbf[:, kt, :] if kt < 4 else ctx_t[:, kt - 4, :]
                            nc.tensor.matmul(
                                ps, wq_sb[:, kt, g * 128 : (g + 1) * 128], rhs,
                                start=(kt == 0), stop=(kt == nkt - 1))
                        qscale = 1.0
                    nc.scalar.activation(
                        q_t[:, g, :], ps, AF.Tanh,
                        bias=qb_sb[:, g : g + 1], scale=qscale)

                if SORDER != "all" and not pair:
                    s_produce(2)
                    s_produce(3)

                if DPRE and t < t_steps - 1:
                    dpre = p_t.tile([128, 1], BF16, tag="dpre")
                    nc.scalar.activation(dpre, qb_sb[:, 0:1],
                                         AF.Derivative_Erf, bias=0.0,
                                         scale=1.0)

                # ---- s consume: state update --------------------------
                h_new = p_h.tile([128, NG, BC], BF16, tag="h")
                h8n = p_h.tile([128, NG, BC], E4, tag="h8")
                if late_r_s is not None:
                    r_s = late_r_s       # recip already emitted post-es
                elif RECIP == "div" or RDT_IS_F32:
                    r_s = den_s          # divide reads den directly
                else:
                    r_s = p_d.tile([128, NG, BC], BF16, tag="rs")

                def s_consume(g):
                    es = es_t[g]
                    if w_pre is not None:
                        v = w_pre[g]
                        w_op = lambda: None     # w already computed
                    else:
                        v = p_v.tile([128, MKS, BC], BF16, tag="v")
                        rsb = _bcast_mid(r_s[:, g, :], MKS)
                        if RECIP == "div":
                            w_op = lambda: nc.vector.tensor_tensor(
                                v, es, rsb, op=mybir.AluOpType.divide)
                        else:
                            w_op = lambda: nc.vector.tensor_mul(v, es, rsb)
                    qb = _bcast_mid(q_t[:, g, :], MKS)
                    if t0:
                        # hh == 0: hh_new = s * q * D
                        w_op()
                        nc.vector.tensor_mul(v, v, qb)
                        nc.vector.tensor_mul(hh[:, g], v, d4)
                    else:
                        u = p_u.tile([128, MKS, BC], BF16, tag="u")
                        if SUB_MODE == "pool_bc":
                            nc.gpsimd.tensor_sub(u, qb, hh[:, g])
                        else:
                            nc.vector.tensor_sub(u, qb, hh[:, g])
                        w_op()
                        nc.vector.tensor_mul(v, v, u)
                        nc.vector.tensor_add(v, v, hh[:, g])
                        nc.vector.tensor_mul(hh[:, g], v, d4)
                    # h = sum_m planes
                    _tree_sum(nc, h_new[:, g, :], hh[:, g], MKS, p_t, "t1h")
                    if H8M:
                        pass                   # single merged copy after g3
                    elif H8ENG == "pool":
                        nc.gpsimd.tensor_copy(h8n[:, g, :], h_new[:, g, :])
                    elif H8ENG == "dve":
                        nc.vector.tensor_copy(h8n[:, g, :], h_new[:, g, :])
                    else:
                        nc.scalar.copy(h8n[:, g, :], h_new[:, g, :])

                def s_consume_wide():
                    """All-g fused state update: 5 wide DVE ops + fused
                    h-tree + single h8 copy."""
                    def wide4(ap3, mid_n):     # [128,NG,BC] -> +stride-0 m dim
                        return AP(ap3.tensor, ap3.offset,
                                  [ap3.ap[0], ap3.ap[1], [0, mid_n], ap3.ap[2]])
                    rsb = wide4(r_s, MKS)
                    qb = wide4(q_t, MKS)
                    d4w = AP(d4.tensor, d4.offset,
                             [d4.ap[0], [0, NG], d4.ap[1], d4.ap[2]])
                    v = p_v.tile([128, NG, MKS, BC], BF16, tag="vw", bufs=1)
                    if t0:
                        nc.vector.tensor_mul(v, es_all, rsb)
                        nc.vector.tensor_mul(v, v, qb)
                        nc.vector.tensor_mul(hh, v, d4w)
                    else:
                        u = p_u.tile([128, NG, MKS, BC], BF16, tag="uw", bufs=1)
                        nc.vector.tensor_sub(u, qb, hh)
                        nc.vector.tensor_mul(v, es_all, rsb)
                        nc.vector.tensor_mul(v, v, u)
                        nc.vector.tensor_add(v, v, hh)
                        nc.vector.tensor_mul(hh, v, d4w)
                    t1 = p_t.tile([128, NG, BC], BF16, tag="t1w")
                    nc.vector.tensor_add(t1, hh[:, :, 0, :], hh[:, :, 1, :])
                    nc.vector.tensor_add(h_new, t1, hh[:, :, 2, :])
                    nc.scalar.copy(h8n, h_new)

                if RECIP != "div" and late_r_s is None:
                    if RECIP1 and SORDER == "all":
                        _recip(nc, r_s, den_s)
                    else:
                        _recip(nc, r_s[:, 0:2, :], den_s[:, 0:2, :])
                        if SORDER == "all":
                            _recip(nc, r_s[:, 2:4, :], den_s[:, 2:4, :])
                if WIDE:
                    assert MKS == 3 and SORDER == "all" and RECIP1
                    s_consume_wide()
                else:
                    s_consume(0)
                    s_consume(1)
                    if RECIP != "div" and SORDER != "all":
                        _recip(nc, r_s[:, 2:4, :], den_s[:, 2:4, :])
                    s_consume(2)
                    s_consume(3)
                    if H8M:
                        nc.scalar.copy(h8n, h_new)

                # ---- output gate (transposed, re-transposed per step) -
                if not OUTDEFER:
                    emit_out_gate(t, h_new)
                elif t == t_steps - 1:
                    emit_out_gate(t, h_new)

                h_prev = h_new
                h8 = h8n

            # ---- final: DMA out ---------------------------------------
            for bs in range(NG):
                nc.sync.dma_start(
                    out=out_d[bs * 128 : (bs + 1) * 128, :, :], in_=o_acc[:, bs, :, :]
                )

    _split_sync_waits(nc, 1)
    return nc


def _host_prep(x, Wr, br, Wq, bq, Ws, bs, Wo, bo, t_steps=T):
    """Shared (weight) tensors + per-core x shards, all pre-permuted."""

    def gmajor_mk(w, mk):
        # (K, U*M) -> (K, NG*mk*128); col (g, m, p), keeping only m < mk
        k = w.shape[0]
        w4 = w.reshape(k, NG, 128, M)[:, :, :, :mk]
        return np.ascontiguousarray(
            w4.transpose(0, 1, 3, 2).reshape(k, NG * mk * 128)
        )

    def gate_weight(w, mk):
        return (gmajor_mk(w, mk) * WSC).astype(NPE4)

    def gmajor_bias(b, mk):
        bm = b.reshape(NG, 128, M)[:, :, :mk]
        return np.ascontiguousarray(
            bm.transpose(1, 0, 2).reshape(128, NG * mk)
        )

    ln_r = np.array([LN_TAU[jt % MKR] for jt in range(NJR)], np.float32)
    ln_s = np.array([LN_TAU[jt % MKS] for jt in range(NJS)], np.float32)

    shared = {
        "wr": gate_weight(Wr, MKR),
        "ws": gate_weight(Ws, MKS),
        "wo": np.ascontiguousarray(Wo).astype(NPBF16),
        "rbias": (gmajor_bias(br, MKR) - ln_r[None, :]).astype(np.float32),
        "sbias": (gmajor_bias(bs, MKS) - ln_s[None, :]).astype(np.float32),
        "qbias": np.ascontiguousarray(bq.reshape(NG, 128).T).astype(np.float32),
    }
    if QFP8:
        shared["wq"] = np.ascontiguousarray(Wq * QSC).astype(NPE4)
    else:
        shared["wq"] = np.ascontiguousarray(Wq).astype(NPBF16)
    xs = []
    for c in range(NCORES):
        xc = x[c * BC : (c + 1) * BC, :t_steps, :]          # (BC, t, F)
        xT = np.ascontiguousarray(xc.transpose(1, 2, 0))
        m = {"xT8": xT.astype(NPBF16).astype(NPE4)}
        if not QFP8:
            m["xT"] = xT.astype(NPBF16)
        xs.append(m)
    return shared, xs


_CACHED = {}


def kernel(x, Wr, br, Wq, bq, Ws, bs, Wo, bo):
    x = np.asarray(x, np.float32)
    Wr = np.asarray(Wr, np.float32)
    br = np.asarray(br, np.float32)
    Wq = np.asarray(Wq, np.float32)
    bq = np.asarray(bq, np.float32)
    Ws = np.asarray(Ws, np.float32)
    bs = np.asarray(bs, np.float32)
    Wo = np.asarray(Wo, np.float32)
    bo = np.asarray(bo, np.float32)

    zero_bias = not (br.any() or bs.any() or bq.any())
    pair = zero_bias if PAIRQ == "auto" else PAIRQ == "1"
    if pair and not zero_bias:
        pair = False           # paired ACTs assume plane-constant bias
    key = ("nc", pair)
    if key not in _CACHED:
        _CACHED[key] = build_program(T, pair=pair)
    nc = _CACHED[key]
    _CACHED["nc"] = nc     # convenience alias (test.py timing harness)

    shared, xs = _host_prep(x, Wr, br, Wq, bq, Ws, bs, Wo, bo)
    in_maps = [dict(shared, **xs[c]) for c in range(NCORES)]
    res = run_bass_kernel_spmd(nc, in_maps, core_ids=list(range(NCORES)))
    out = np.concatenate([res.results[c]["out"] for c in range(NCORES)], axis=0)
    return (out + bo[None, None, :]).astype(np.float32)
